# revision 29
# baseline (speedup 1.0000x reference)
"""Bass/Trainium2 kernel for nn_KernelEdges (gnn_message_passing).

Computes A = exp((g_i + g_j - 2*Xf@Xf.T)/sigma^2) with zeroed diagonal,
broadcast to all B batch slots, where Xf = X.transpose(1,0,2).reshape(N, B*d).

Sharding: rows of the NxN pairwise matrix are split across 8 NeuronCores
(256 rows each).  The batch dim of the output is a pure replication of the
same [N, N] matrix, so each core writes only its unique [N/8, N] tile and
the host broadcasts to the B batch slots (as the reference itself does).

Each core receives a column-ROLLED copy of XT = Xf.T [B*d, N] so that its
own 256 columns sit at rolled positions 0..255; the matmul LHS (stationary
operand) is then a fixed slice of the streamed xt tile and no separate
lhst input is needed.  The host un-rolls the output columns after gather.

Per-core device work, per psum chain (mt, g) over column group g:
  psum = (-1/2*ones).T @ g_row[g]                        (rank-1: -g_j/2)
       + sum_q xt_q[:, mt_cols].T @ xt_q[:, g_cols]      (Gram matrix)
  A    = exp(-2/sigma^2 * psum + g_i/sigma^2)            (ACT, bias/row)
  DMA the [128, w] bf16 piece to its slot of the [N/8, N] tile.

Schedule rationale (tuned against perfetto traces):
  - Each HWDGE ring is a serial ~150-430GB/s pipe (throughput rises with
    row size) with ~2.1us trigger->data fill after idle, and a store can
    only be triggered after its ACT completes — so the LAST store always
    pays ACT + trigger + fill + transfer as pure tail.
  - Columns split into ASYMMETRIC groups [1024, 512, 512], streamed
    group-major: chains stop in three waves, the ACT stream drains
    progressively, and the final tail is a narrow 0.69us ACT plus a
    small [128,512] store instead of a wide 1.11us ACT + double-size
    store.  Stores spread over all three rings so they never serialize
    behind each other.
  - The chip power-throttles the PE to ~50% util for 25-40% of the run,
    so schedules that add work (warmup matmuls) or more concurrent
    engine activity measured consistently WORSE; the g_j rank-1 seeds
    stay on the PE (gpsimd's ucode partition_broadcast is a barrier,
    and a DVE multiply would lengthen the tail).

The diagonal is zeroed on the host (2K elements) after the gather.
"""

import numpy as np

B, N, D = 8, 2048, 64
NCORES = 8
R = N // NCORES          # 256 rows per core
KD = B * D               # 512 contraction dim
NB = 512                 # n-block (one PSUM bank of fp32)
NMT = R // 128           # 2 m-tiles per core
NQ = KD // 128           # 4 k-tiles

# asymmetric column groups (offset, width): psum chains are [128, w], and
# group widths shrink toward the end so the tail ACT+store is small
COLGROUPS = [(0, 1024), (1024, 512), (1536, 512)]

MM_MODE = "bf16"         # matmul operand dtype ("bf16" | "f32r")
OUT_BF16 = True          # store A as bf16, upcast on host

# input piece (g, q) -> ring.  The sync ring carries the main stream in
# order; the two q3 pieces of the small groups ride the scalar ring early
# so the last chains are gated by sync's q2 pieces, not a serial tail.
PIECE_QUEUE = {
    (g, q): "sync" for g in range(len(COLGROUPS)) for q in range(NQ)
}
PIECE_QUEUE[(1, 3)] = "scalar"
PIECE_QUEUE[(2, 3)] = "scalar"

# store piece (g, mt) -> ring: group-0 stores chain behind the input on
# the busy sync ring (no refill); later stores go to idle rings so they
# run in parallel; the final store rides scalar right after its ACT.
STORE_QUEUE = {
    (0, 0): "sync", (0, 1): "sync",
    (1, 0): "gpsimd", (1, 1): "gpsimd",
    (2, 0): "gpsimd", (2, 1): "scalar",
}


def _build_program(inv_s2):
    import concourse.bass as bass
    import concourse.tile as tile
    from concourse import bacc, mybir

    f32 = mybir.dt.float32
    mm_dt = mybir.dt.bfloat16 if MM_MODE == "bf16" else mybir.dt.float32r
    out_dt = mybir.dt.bfloat16 if OUT_BF16 else f32

    nc = bacc.Bacc(
        "TRN2", target_bir_lowering=False, debug=False, num_devices=NCORES
    )

    GK = 2 if MM_MODE == "bf16" else 1  # g carried as hi+lo rows in bf16

    xt_d = nc.dram_tensor("xt", [KD, N], mm_dt, kind="ExternalInput").ap()
    bias_d = nc.dram_tensor("bias", [128, NMT], f32, kind="ExternalInput").ap()
    grow_d = nc.dram_tensor("grow", [GK, N], mm_dt, kind="ExternalInput").ap()
    out_d = nc.dram_tensor(
        "out", [NMT * 128, N], out_dt, kind="ExternalOutput"
    ).ap()

    with tile.TileContext(nc) as tc:
        with (
            tc.tile_pool(name="persist", bufs=1) as persist,
            tc.tile_pool(name="apool", bufs=1) as apool,
            tc.tile_pool(name="psum", bufs=1, space="PSUM") as pspool,
        ):
            # ---- small loads ----
            neg_half = persist.tile([GK, 128], mm_dt, name="neg_half")
            if MM_MODE == "bf16":
                nc.gpsimd.memset(
                    neg_half[:].bitcast(mybir.dt.uint16), 0xBF00
                )
            else:
                nc.gpsimd.memset(
                    neg_half[:].bitcast(mybir.dt.uint32), 0xBF000000
                )
            # grow gates the rank-1 seeds (the first PE work): first on
            # the otherwise-idle gpsimd ring
            grow_sb = persist.tile([GK, N], mm_dt, name="grow")
            nc.gpsimd.dma_start(grow_sb[:], grow_d[:])

            bias_sb = persist.tile([128, NMT], f32, name="bias")
            nc.scalar.dma_start(bias_sb[:], bias_d[:])

            # ---- xt piece loads, group-major ----
            xt_sb = [
                persist.tile([128, N], mm_dt, name=f"xt{q}")
                for q in range(NQ)
            ]
            engines = {
                "sync": nc.sync, "scalar": nc.scalar, "gpsimd": nc.gpsimd
            }
            for g, (off, w) in enumerate(COLGROUPS):
                for q in range(NQ):
                    engines[PIECE_QUEUE[(g, q)]].dma_start(
                        xt_sb[q][:, off:off + w],
                        xt_d[q * 128:(q + 1) * 128, off:off + w],
                    )

            # ---- compute + store ----
            ps = {
                (mt, g): pspool.tile([128, w], f32, name=f"ps{mt}{g}")
                for g, (off, w) in enumerate(COLGROUPS)
                for mt in range(NMT)
            }
            a_sb = {
                mt: apool.tile([128, N], out_dt, name=f"a{mt}")
                for mt in range(NMT)
            }
            # group-major matmuls in piece arrival order (PE is in-order);
            # rank-1 seeds interleave per-group so they hide in DMA-wait
            # gaps.  LHS is the core's own 256 rolled columns, a slice of
            # the group-0 pieces which always arrive first.
            for g, (off, w) in enumerate(COLGROUPS):
                nsub = w // NB
                for mt in range(NMT):
                    for s in range(nsub):
                        nc.tensor.matmul(
                            ps[mt, g][:, s * NB:(s + 1) * NB],
                            neg_half[:],
                            grow_sb[:, off + s * NB:off + (s + 1) * NB],
                            start=True,
                            stop=False,
                        )
                for q in range(NQ):
                    for mt in range(NMT):
                        for s in range(nsub):
                            nc.tensor.matmul(
                                ps[mt, g][:, s * NB:(s + 1) * NB],
                                xt_sb[q][:, mt * 128:(mt + 1) * 128],
                                xt_sb[q][
                                    :, off + s * NB:off + (s + 1) * NB
                                ],
                                start=False,
                                stop=(q == NQ - 1),
                            )
            # ACT + store chase the chains in stop order
            for g, (off, w) in enumerate(COLGROUPS):
                for mt in range(NMT):
                    nc.scalar.activation(
                        a_sb[mt][:, off:off + w],
                        ps[mt, g][:],
                        mybir.ActivationFunctionType.Exp,
                        bias=bias_sb[:, mt:mt + 1],
                        scale=-2.0 * inv_s2,
                    )
                    engines[STORE_QUEUE[(g, mt)]].dma_start(
                        out_d[mt * 128:(mt + 1) * 128, off:off + w],
                        a_sb[mt][:, off:off + w],
                    )

    nc.compile()
    return nc


def _prepare(X, log_sigma):
    """Host prep: returns (inv_s2, in_maps) for run_bass_kernel_spmd."""
    import ml_dtypes

    X = np.ascontiguousarray(X, dtype=np.float32)
    assert X.shape == (B, N, D), X.shape

    sigma = float(np.exp(np.float32(log_sigma)))
    inv_s2 = 1.0 / (sigma * sigma)

    # XT[b*D+f, n] = X[b, n, f]
    XT = np.ascontiguousarray(X.transpose(0, 2, 1).reshape(KD, N))
    g = np.einsum("kn,kn->n", XT, XT).astype(np.float32)  # [N]

    mm_np = ml_dtypes.bfloat16 if MM_MODE == "bf16" else np.float32
    XTm = XT.astype(mm_np)

    in_maps = []
    for c in range(NCORES):
        r0 = c * R
        # roll columns so this core's own block is at rolled cols 0..R-1
        xt_t = np.ascontiguousarray(np.roll(XTm, -r0, axis=1))

        gr = np.roll(g, -r0)
        bias_np = np.empty((128, NMT), dtype=np.float32)
        for mt in range(NMT):
            bias_np[:, mt] = g[r0 + mt * 128: r0 + (mt + 1) * 128] * inv_s2
        if MM_MODE == "bf16":
            g_hi = gr.astype(ml_dtypes.bfloat16)
            g_lo = (gr - g_hi.astype(np.float32)).astype(ml_dtypes.bfloat16)
            grow_np = np.ascontiguousarray(np.stack([g_hi, g_lo]))
        else:
            grow_np = np.ascontiguousarray(gr[None, :])
        in_maps.append({"xt": xt_t, "bias": bias_np, "grow": grow_np})
    return inv_s2, in_maps


def kernel(X, log_sigma):
    from concourse.bass_utils import run_bass_kernel_spmd

    inv_s2, in_maps = _prepare(X, log_sigma)
    nc = _build_program(inv_s2)
    res = run_bass_kernel_spmd(nc, in_maps, list(range(NCORES)))

    A = np.empty((N, N), dtype=np.float32)
    for c in range(NCORES):
        r0 = c * R
        t = np.asarray(res.results[c]["out"]).astype(np.float32)  # [R, N]
        # un-roll columns back to global positions
        A[r0:r0 + R, :] = np.roll(t, r0, axis=1)
    idx = np.arange(N)
    A[idx, idx] = 0.0
    out = np.empty((B, N, N), dtype=np.float32)
    out[:] = A[None, :, :]
    return out
